# revision 5
# baseline (speedup 1.0000x reference)
"""GQA causal self-attention on 8 Trainium2 NeuronCores.

Problem: B=2, T=2048, C=2048, H=16 query heads, HKV=4 kv heads, HD=128.
Sharding: core (b, g) for b in {0,1}, g in {0..3} owns batch b, kv head g,
and the 4 query heads h with h % 4 == g (reference's _expand_kv maps query
head h -> kv head h % HKV).  Each core computes its heads' attention output
and a partial output projection (its 512 rows of Wp); the host sums the 4
partials per batch and adds bp.  No cross-core communication on device.

All DRAM inputs are host-repacked to [128 partitions, ...contiguous]
layouts so every DMA chunk is >=4KB contiguous per partition (fast issue,
fast transfer).

Device math per core (all matmuls fp16 operands, fp32 PSUM accumulation):
  qT[d, t] = Wq_g.T @ x_b.T      (x is fed pre-transposed from host)
  kT[d, t] = Wk_g.T @ x_b.T
  v[t, d]  = x_b @ Wv_g          (lhsT = xT tiles)
  ST[j, i] = kT_j . qT_i         (j keys on partitions, i queries free)
  A = exp(ST / sqrt(HD)); causal: off-diagonal key tiles computed full
      width, the 4 diagonal key tiles of each i-tile computed only on
      their live query ranges (512/384/256/128 wide) with a tril mask on
      the single triangular 128-block each
  den[*, i] = ones-matmul over gpsimd-accumulated row sums
  yT[d, i] = (sum_j v[j, d] A[j, i]) / den[i]
  out[i, o] += yT.T @ Wp_g       (partial, fp16; host sums over g)
"""

import math
import os
from contextlib import ExitStack

import numpy as np

import concourse.bass as bass
import concourse.mybir as mybir
import concourse.tile as tile
from concourse import bacc, bass_utils

# The axon trace path needs antenv.axon_hooks; if the environment requests
# tracing but lacks the hook module, force tracing off instead of crashing.
if os.environ.get("BASS_TRACE"):
    try:
        import antenv.axon_hooks  # noqa: F401
    except ImportError:
        os.environ["BASS_NEVER_TRACE"] = "1"

# Problem shapes (hardcoded per contest rules).
B, T, C = 2, 2048, 2048
H, G = 16, 4
HKV = H // G          # 4 kv heads
HD = C // H           # 128 head dim
P = 128               # partitions
NH = H // HKV         # 4 local query heads per core
KT = C // P           # 16 contraction tiles for projections
TW = 512              # token tile width (matmul free dim)
NT = T // TW          # 4 token tiles
JTN = T // P          # 16 key tiles of 128
SCALE = 1.0 / math.sqrt(HD)

FP = mybir.dt.float16
F32 = mybir.dt.float32

_CACHE = {}

# Set by kernel() after each run: bass_utils.BassKernelResults.
LAST_RESULT = None


def _build_bass():
    nc = bacc.Bacc("TRN2")

    # Host-packed layouts: partition dim first, then contiguous payload.
    xt = nc.dram_tensor("xt", [P, NT, KT, TW], FP, kind="ExternalInput")
    wq = nc.dram_tensor("wq", [P, KT, NH * HD], FP, kind="ExternalInput")
    wk = nc.dram_tensor("wk", [P, KT, HD], FP, kind="ExternalInput")
    wv = nc.dram_tensor("wv", [P, KT, HD], FP, kind="ExternalInput")
    wp = nc.dram_tensor("wp", [P, NH, C], FP, kind="ExternalInput")
    bq = nc.dram_tensor("bq", [P, NH], F32, kind="ExternalInput")
    bk = nc.dram_tensor("bk", [P, 1], F32, kind="ExternalInput")
    bv = nc.dram_tensor("bv", [HD], F32, kind="ExternalInput")
    mask = nc.dram_tensor("mask", [P, 2, P], FP, kind="ExternalInput")
    out = nc.dram_tensor("out", [T, C], FP, kind="ExternalOutput")

    out_r = out.ap().rearrange("(io p) o -> p io o", p=P)     # [128,16,2048]

    with tile.TileContext(nc) as tc, ExitStack() as ctx:
        consts = ctx.enter_context(tc.tile_pool(name="consts", bufs=1))
        xpool = ctx.enter_context(tc.tile_pool(name="xpool", bufs=2))
        espool = ctx.enter_context(tc.tile_pool(name="espool", bufs=4))
        mpool = ctx.enter_context(tc.tile_pool(name="mpool", bufs=2))
        opool = ctx.enter_context(tc.tile_pool(name="opool", bufs=2))
        # PSUM (8 banks): ps_s 2x[128,2,512] (4) for S pairs + q/k proj,
        # ps_y 1x[128,512] (1), ps_d 1x[128,512] (1) also l2-S and v-proj,
        # ps_o 1x[128,2,512] (2) for out-proj halves.
        ps_s = ctx.enter_context(tc.tile_pool(name="ps_s", bufs=2, space="PSUM"))
        ps_y = ctx.enter_context(tc.tile_pool(name="ps_y", bufs=1, space="PSUM"))
        ps_d = ctx.enter_context(tc.tile_pool(name="ps_d", bufs=1, space="PSUM"))
        ps_o = ctx.enter_context(tc.tile_pool(name="ps_o", bufs=1, space="PSUM"))

        # Weights needed first, loaded in k-chunks interleaved with the first
        # x tile so the first q matmul can start as early as possible.  x
        # rides the sync queue; weights ride the gpsimd queue so the startup
        # burst is split across two DMA rings.
        KC = 4  # k-chunks per load
        wq_sb = consts.tile([P, KT, NH * HD], FP)
        wk_sb = consts.tile([P, KT, HD], FP)
        wv_sb = consts.tile([P, KT, HD], FP)
        mask_sb = consts.tile([P, 2, P], FP)
        nc.gpsimd.dma_start(out=mask_sb, in_=mask.ap())
        xtile0 = xpool.tile([P, KT, TW], FP, tag="xt", name="xtile0")
        for c4 in range(KC):
            ks = slice(c4 * (KT // KC), (c4 + 1) * (KT // KC))
            nc.sync.dma_start(out=xtile0[:, ks], in_=xt.ap()[:, 0, ks])
            nc.gpsimd.dma_start(out=wq_sb[:, ks], in_=wq.ap()[:, ks])
            nc.gpsimd.dma_start(out=wk_sb[:, ks], in_=wk.ap()[:, ks])
            nc.gpsimd.dma_start(out=wv_sb[:, ks], in_=wv.ap()[:, ks])
        bq_sb = consts.tile([P, NH], F32)
        nc.gpsimd.dma_start(out=bq_sb, in_=bq.ap())
        bk_sb = consts.tile([P, 1], F32)
        nc.gpsimd.dma_start(out=bk_sb, in_=bk.ap())
        # bv broadcast across partitions (DRAM source allows partition step 0).
        bv_bc = consts.tile([P, HD], F32)
        bv_ap = bass.AP(tensor=bv.ap().tensor, offset=0, ap=[[0, P], [1, HD]])
        nc.gpsimd.dma_start(out=bv_bc, in_=bv_ap)
        ones_sb = consts.tile([P, P], FP)
        nc.vector.memset(ones_sb, 1.0)
        dummy_sb = consts.tile([P, TW], FP)
        nc.vector.memset(dummy_sb, 0.0)

        # PE warm-up: HAM un-throttles (1.2 -> 2.4 GHz) after ~3.4us of
        # sustained matmul activity.  Run throwaway matmuls while the input
        # DMAs land so the real matmuls start at full clock.
        ps_warm = ps_o.tile([P, 2, TW], F32, tag="pso", name="ps_warm")
        for w in range(10):
            nc.tensor.matmul(
                ps_warm[:, w % 2, :],
                lhsT=ones_sb,
                rhs=dummy_sb,
                start=True,
                stop=True,
            )

        # Persistent activations.
        qT = consts.tile([P, NH, T], FP)       # [d, h, i]
        kT = consts.tile([P, T], FP)           # [d, j]
        v_sb = consts.tile([P, JTN, HD], FP)   # [j_in, j_tile, d]
        yT = consts.tile([P, NH, T], FP)       # [d, h, i]

        # ---- Projections ----
        # Stream the contraction dim: per 4-k chunk, feed all accumulators
        # (4 q heads, k, 4 v token blocks) so compute starts as soon as the
        # first chunk of wq/xt lands instead of after the full 4MB.
        for n in range(NT):
            if n == 0:
                xtile = xtile0
            else:
                xtile = xpool.tile([P, KT, TW], FP, tag="xt", name=f"xtile{n}")
                for c4 in range(KC):
                    ks = slice(c4 * (KT // KC), (c4 + 1) * (KT // KC))
                    nc.sync.dma_start(out=xtile[:, ks], in_=xt.ap()[:, n, ks])
            psq01 = ps_s.tile([P, 2, TW], F32, tag="pss", name=f"psq01_{n}")
            psq23 = ps_s.tile([P, 2, TW], F32, tag="pss", name=f"psq23_{n}")
            psk = ps_y.tile([P, TW], F32, tag="psy", name=f"psk_{n}")
            for k in range(KT):
                st = k == 0
                sp = k == KT - 1
                for h in range(NH):
                    tgt = psq01 if h < 2 else psq23
                    nc.tensor.matmul(
                        tgt[:, h % 2, :],
                        lhsT=wq_sb[:, k, h * HD:(h + 1) * HD],
                        rhs=xtile[:, k, :],
                        start=st,
                        stop=sp,
                    )
                nc.tensor.matmul(
                    psk, lhsT=wk_sb[:, k, :], rhs=xtile[:, k, :], start=st, stop=sp
                )
            for h in range(NH):
                tgt = psq01 if h < 2 else psq23
                nc.vector.tensor_scalar(
                    out=qT[:, h, n * TW:(n + 1) * TW],
                    in0=tgt[:, h % 2, :],
                    scalar1=bq_sb[:, h:h + 1],
                    scalar2=None,
                    op0=mybir.AluOpType.add,
                )
            nc.vector.tensor_scalar(
                out=kT[:, n * TW:(n + 1) * TW],
                in0=psk,
                scalar1=bk_sb,
                scalar2=None,
                op0=mybir.AluOpType.add,
            )
            # v-projection: DMA-independent by now (q/k streamed the whole
            # xtile); two accumulators per ps_o tile, one bank each.
            for jp in range(TW // P // 2):
                psv = ps_o.tile([P, 2, TW], F32, tag="pso", name=f"psv_{n}_{jp}")
                for u in range(2):
                    js = jp * 2 + u
                    for k in range(KT):
                        nc.tensor.matmul(
                            psv[:, u, :HD],
                            lhsT=xtile[:, k, js * P:(js + 1) * P],
                            rhs=wv_sb[:, k, :],
                            start=(k == 0),
                            stop=(k == KT - 1),
                        )
                for u in range(2):
                    jt = n * (TW // P) + jp * 2 + u
                    nc.vector.tensor_tensor(
                        out=v_sb[:, jt, :],
                        in0=psv[:, u, :HD],
                        in1=bv_bc,
                        op=mybir.AluOpType.add,
                    )

        # Weights for the out-projection: load after projection work is
        # queued, on the gpsimd queue (idle once the small weights landed).
        wp_sb = consts.tile([P, NH, C], FP)
        nc.gpsimd.dma_start(out=wp_sb, in_=wp.ap())

        # ---- Attention with interleaved output projection ----
        # Out-proj for i-tile it is emitted *split* between the attention
        # heads of i-tile it+1: half right after the diagonal S matmuls
        # (covering the diag exp latency), half after the den matmul.  The
        # row-sum accumulation runs on the otherwise-idle gpsimd engine.
        def out_proj_half(ic, half, osb, pool=ps_o, ptag="pso"):
            pso = pool.tile([P, 2, TW], F32, tag=ptag, name=f"pso_{ic}_{half}")
            for h in range(NH):
                for u in range(2):
                    ot = half * 2 + u
                    nc.tensor.matmul(
                        pso[:, u, :],
                        lhsT=yT[:, h, ic * P:(ic + 1) * P],
                        rhs=wp_sb[:, h, ot * TW:(ot + 1) * TW],
                        start=(h == 0),
                        stop=(h == NH - 1),
                    )
            hsl = slice(half * 2 * TW, (half + 1) * 2 * TW)
            nc.any.tensor_copy(out=osb[:, hsl], in_=pso)
            if half == 1:
                nc.sync.dma_start(out=out_r[:, ic, :], in_=osb)

        for it in range(NT):
            isl = slice(it * TW, (it + 1) * TW)
            noff = 2 * it          # full-width off-diagonal key-tile pairs
            for h in range(NH):
                ic = (it - 1) * (TW // P) + h   # out-proj chunk to interleave
                osb = None
                if it > 0:
                    osb = opool.tile([P, C], FP, tag="osb", name=f"osb_{ic}")
                psy = ps_y.tile([P, TW], F32, tag="psy", name=f"psy_{it}_{h}")
                acc = mpool.tile([P, 2, TW], FP, tag="acc", name=f"acc_{it}_{h}")
                # --- off-diagonal pairs: full 512-wide, unmasked ---
                for pr in range(noff):
                    jt0 = 2 * pr
                    pss = ps_s.tile(
                        [P, 2, TW], F32, tag="pss", name=f"pss_{it}_{h}_{pr}"
                    )
                    for u in range(2):
                        nc.tensor.matmul(
                            pss[:, u, :],
                            lhsT=kT[:, (jt0 + u) * P:(jt0 + u + 1) * P],
                            rhs=qT[:, h, isl],
                            start=True,
                            stop=True,
                        )
                    es = espool.tile([P, 2, TW], FP, tag="es")
                    nc.scalar.activation(
                        out=es,
                        in_=pss,
                        func=mybir.ActivationFunctionType.Exp,
                        scale=SCALE,
                    )
                    if pr == 0:
                        nc.gpsimd.tensor_copy(out=acc, in_=es)
                    else:
                        nc.gpsimd.tensor_tensor(
                            out=acc, in0=acc, in1=es, op=mybir.AluOpType.add
                        )
                    for u in range(2):
                        nc.tensor.matmul(
                            psy,
                            lhsT=v_sb[:, jt0 + u, :],
                            rhs=es[:, u, :],
                            start=(jt0 + u == 0),
                            stop=False,
                        )
                # --- diagonal: 4 key tiles, live query ranges only ---
                # diagA psum pair: u0 <- l0 (full 512), u1 <- l1 (384-wide,
                # queries [128:512)) packed at [0:384) ++ l3 (128-wide,
                # queries [384:512)) at [384:512).  l2 (256-wide, queries
                # [256:512)) rides the ps_d bank ahead of the den matmul.
                jd = 4 * it        # first diagonal key tile
                diagA = ps_s.tile(
                    [P, 2, TW], F32, tag="pss", name=f"diagA_{it}_{h}"
                )
                psd_s = ps_d.tile([P, TW], F32, tag="psd", name=f"psl2_{it}_{h}")
                nc.tensor.matmul(
                    diagA[:, 0, :],
                    lhsT=kT[:, jd * P:(jd + 1) * P],
                    rhs=qT[:, h, isl],
                    start=True,
                    stop=True,
                )
                nc.tensor.matmul(
                    diagA[:, 1, 0:384],
                    lhsT=kT[:, (jd + 1) * P:(jd + 2) * P],
                    rhs=qT[:, h, it * TW + P:(it + 1) * TW],
                    start=True,
                    stop=True,
                )
                nc.tensor.matmul(
                    diagA[:, 1, 384:512],
                    lhsT=kT[:, (jd + 3) * P:(jd + 4) * P],
                    rhs=qT[:, h, it * TW + 3 * P:(it + 1) * TW],
                    start=True,
                    stop=True,
                )
                nc.tensor.matmul(
                    psd_s[:, 0:256],
                    lhsT=kT[:, (jd + 2) * P:(jd + 3) * P],
                    rhs=qT[:, h, it * TW + 2 * P:(it + 1) * TW],
                    start=True,
                    stop=True,
                )
                esd = espool.tile([P, 2, TW], FP, tag="es", name=f"esd_{it}_{h}")
                nc.scalar.activation(
                    out=esd,
                    in_=diagA,
                    func=mybir.ActivationFunctionType.Exp,
                    scale=SCALE,
                )
                es2 = espool.tile([P, 2, TW], FP, tag="es", name=f"es2_{it}_{h}")
                nc.scalar.activation(
                    out=es2[:, 0, 0:256],
                    in_=psd_s[:, 0:256],
                    func=mybir.ActivationFunctionType.Exp,
                    scale=SCALE,
                )
                # PE filler while ACT computes the diagonal exps.
                if it > 0:
                    out_proj_half(ic, 0, osb)
                # tril masks: l0 triangle at u0[0:128), l1 triangle at
                # u1[0:128) (one paired op), l3 triangle at u1[384:512),
                # l2 triangle at es2[0:128).
                nc.vector.tensor_mul(
                    esd[:, :, 0:P], esd[:, :, 0:P], mask_sb
                )
                nc.vector.tensor_mul(
                    esd[:, 1, 384:512], esd[:, 1, 384:512], mask_sb[:, 0, :]
                )
                nc.vector.tensor_mul(
                    es2[:, 0, 0:P], es2[:, 0, 0:P], mask_sb[:, 0, :]
                )
                # accumulate row sums (per-query) into the pair acc (gpsimd)
                if it == 0:
                    nc.gpsimd.tensor_copy(out=acc[:, 0, :], in_=esd[:, 0, :])
                    nc.gpsimd.memset(acc[:, 1, 0:P], 0.0)
                    nc.gpsimd.tensor_copy(
                        out=acc[:, 1, P:TW], in_=esd[:, 1, 0:384]
                    )
                else:
                    nc.gpsimd.tensor_tensor(
                        out=acc[:, 0, :], in0=acc[:, 0, :], in1=esd[:, 0, :],
                        op=mybir.AluOpType.add,
                    )
                    nc.gpsimd.tensor_tensor(
                        out=acc[:, 1, P:TW], in0=acc[:, 1, P:TW],
                        in1=esd[:, 1, 0:384], op=mybir.AluOpType.add,
                    )
                nc.gpsimd.tensor_tensor(
                    out=acc[:, 0, 3 * P:TW], in0=acc[:, 0, 3 * P:TW],
                    in1=esd[:, 1, 384:512], op=mybir.AluOpType.add,
                )
                nc.gpsimd.tensor_tensor(
                    out=acc[:, 1, 2 * P:TW], in0=acc[:, 1, 2 * P:TW],
                    in1=es2[:, 0, 0:256], op=mybir.AluOpType.add,
                )
                # --- AV for the diagonal ---
                # it==0: l0 (full width) first with start=True so no matmul
                # needs a sub-range start; the last one (l2) carries stop.
                # it>0: psy already initialized by off-diag jt=0; l0 goes
                # last, full width, with stop.
                if it == 0:
                    nc.tensor.matmul(
                        psy,
                        lhsT=v_sb[:, jd, :],
                        rhs=esd[:, 0, :],
                        start=True,
                        stop=False,
                    )
                nc.tensor.matmul(
                    psy[:, P:TW],
                    lhsT=v_sb[:, jd + 1, :],
                    rhs=esd[:, 1, 0:384],
                    start=False,
                    stop=False,
                )
                nc.tensor.matmul(
                    psy[:, 3 * P:TW],
                    lhsT=v_sb[:, jd + 3, :],
                    rhs=esd[:, 1, 384:512],
                    start=False,
                    stop=False,
                )
                nc.tensor.matmul(
                    psy[:, 2 * P:TW],
                    lhsT=v_sb[:, jd + 2, :],
                    rhs=es2[:, 0, 0:256],
                    start=False,
                    stop=(it == 0),
                )
                if it > 0:
                    nc.tensor.matmul(
                        psy,
                        lhsT=v_sb[:, jd, :],
                        rhs=esd[:, 0, :],
                        start=False,
                        stop=True,
                    )
                # --- denominator: fold pair slots, one ones-matmul ---
                accs = mpool.tile([P, TW], FP, tag="accs", name=f"accs_{it}_{h}")
                nc.gpsimd.tensor_tensor(
                    out=accs, in0=acc[:, 0, :], in1=acc[:, 1, :],
                    op=mybir.AluOpType.add,
                )
                psd = ps_d.tile([P, TW], F32, tag="psd", name=f"psd_{it}_{h}")
                nc.tensor.matmul(
                    psd, lhsT=ones_sb, rhs=accs, start=True, stop=True
                )
                if it > 0:
                    out_proj_half(ic, 1, osb)
                rb = mpool.tile([P, TW], F32, tag="rb")
                nc.vector.reciprocal_approx_fast(out=rb, in_=psd)
                nc.vector.tensor_mul(yT[:, h, isl], psy, rb)
        # Tail chunks: attention is done, so the S-pair pool (2 slots) is free
        # and gives half-to-half pipelining.
        for h in range(NH):
            ic = (NT - 1) * (TW // P) + h
            osb = opool.tile([P, C], FP, tag="osb", name=f"osb_{ic}")
            for half in range(2):
                out_proj_half(ic, half, osb, pool=ps_s, ptag="pss")

    nc.compile()
    return nc


def _causal_mask_tiles():
    # [128, 2, 128] tril (key_in_tile <= query_in_block), both slots equal.
    j = np.arange(P)[:, None, None]
    i = np.arange(P)[None, None, :]
    return np.broadcast_to(j <= i, (P, 2, P)).astype(np.float16)


def kernel(x, Wkv, bkv, Wq, bq, Wp, bp):
    global LAST_RESULT
    x = np.asarray(x, np.float32)
    Wkv = np.asarray(Wkv, np.float32)
    bkv = np.asarray(bkv, np.float32)
    Wq = np.asarray(Wq, np.float32)
    bq = np.asarray(bq, np.float32)
    Wp = np.asarray(Wp, np.float32)
    bp = np.asarray(bp, np.float32)

    if "nc" not in _CACHE:
        _CACHE["nc"] = _build_bass()
    nc = _CACHE["nc"]

    mask = _causal_mask_tiles()
    CG = C // G  # 512 columns per kv head in the k/v halves of Wkv

    in_maps = []
    for b in range(B):
        # xt packed: [p, n, ko, tw] = x[b].T[ko*128+p, n*512+tw]
        xtb = x[b].T.astype(np.float16).reshape(KT, P, NT, TW)
        xt_packed = np.ascontiguousarray(xtb.transpose(1, 2, 0, 3))
        for g in range(HKV):
            heads = [g + HKV * u for u in range(NH)]  # h % HKV == g
            wq_g = np.concatenate(
                [Wq[:, h * HD:(h + 1) * HD] for h in heads], axis=1
            ).astype(np.float16)
            wq_p = np.ascontiguousarray(
                wq_g.reshape(KT, P, NH * HD).transpose(1, 0, 2)
            )
            bq_g = np.concatenate([bq[h * HD:(h + 1) * HD] for h in heads])
            bq_p = np.ascontiguousarray(
                bq_g.reshape(NH, P).T.astype(np.float32)
            )
            wp_g = np.concatenate(
                [Wp[h * HD:(h + 1) * HD, :] for h in heads], axis=0
            ).astype(np.float16)
            wp_p = np.ascontiguousarray(wp_g.reshape(NH, P, C).transpose(1, 0, 2))
            wk_g = Wkv[:, g * HD:(g + 1) * HD].astype(np.float16)
            wk_p = np.ascontiguousarray(wk_g.reshape(KT, P, HD).transpose(1, 0, 2))
            wv_g = Wkv[:, CG + g * HD:CG + (g + 1) * HD].astype(np.float16)
            wv_p = np.ascontiguousarray(wv_g.reshape(KT, P, HD).transpose(1, 0, 2))
            bk_g = np.ascontiguousarray(
                bkv[g * HD:(g + 1) * HD].reshape(P, 1), np.float32
            )
            bv_g = np.ascontiguousarray(
                bkv[CG + g * HD:CG + (g + 1) * HD], np.float32
            )
            in_maps.append(
                {
                    "xt": xt_packed,
                    "wq": wq_p,
                    "wk": wk_p,
                    "wv": wv_p,
                    "wp": wp_p,
                    "bq": bq_p,
                    "bk": bk_g,
                    "bv": bv_g,
                    "mask": mask,
                }
            )

    res = bass_utils.run_bass_kernel_spmd(nc, in_maps, core_ids=list(range(B * HKV)))
    LAST_RESULT = res

    out = np.zeros((B, T, C), np.float32)
    for b in range(B):
        acc = np.zeros((T, C), np.float32)
        for g in range(HKV):
            acc += res.results[b * HKV + g]["out"]
        out[b] = acc + bp[None, :]
    return out


# revision 6
# speedup vs baseline: 1.3723x; 1.3723x over previous
"""GQA causal self-attention on 8 Trainium2 NeuronCores.

Problem: B=2, T=2048, C=2048, H=16 query heads, HKV=4 kv heads, HD=128.
Sharding: core (b, g) for b in {0,1}, g in {0..3} owns batch b, kv head g,
and the 4 query heads h with h % 4 == g (reference's _expand_kv maps query
head h -> kv head h % HKV).  Each core computes its heads' attention output
and a partial output projection (its 512 rows of Wp); the host sums the 4
partials per batch and adds bp.  No cross-core communication on device.

All DRAM inputs are host-repacked to [128 partitions, ...contiguous]
layouts so every DMA chunk is >=4KB contiguous per partition (fast issue,
fast transfer).

Device math per core (all matmuls fp16 operands, fp32 PSUM accumulation):
  qT[d, t] = Wq_g.T @ x_b.T      (x is fed pre-transposed from host)
  kT[d, t] = Wk_g.T @ x_b.T
  v[t, d]  = x_b @ Wv_g          (lhsT = xT tiles)
  ST[j, i] = kT_j . qT_i         (j keys on partitions, i queries free)
  A = exp(ST / sqrt(HD)); causal: off-diagonal key tiles computed full
      width, the 4 diagonal key tiles of each i-tile computed only on
      their live query ranges (512/384/256/128 wide) with a tril mask on
      the single triangular 128-block each
  den[*, i] = ones-matmul over gpsimd-accumulated row sums
  yT[d, i] = (sum_j v[j, d] A[j, i]) / den[i]
  out[i, o] += yT.T @ Wp_g       (partial, fp16; host sums over g)
"""

import math
import os
from contextlib import ExitStack

import numpy as np

import concourse.bass as bass
import concourse.mybir as mybir
import concourse.tile as tile
from concourse import bacc, bass_utils

# The axon trace path needs antenv.axon_hooks; if the environment requests
# tracing but lacks the hook module, force tracing off instead of crashing.
if os.environ.get("BASS_TRACE"):
    try:
        import antenv.axon_hooks  # noqa: F401
    except ImportError:
        os.environ["BASS_NEVER_TRACE"] = "1"

# Problem shapes (hardcoded per contest rules).
B, T, C = 2, 2048, 2048
H, G = 16, 4
HKV = H // G          # 4 kv heads
HD = C // H           # 128 head dim
P = 128               # partitions
NH = H // HKV         # 4 local query heads per core
KT = C // P           # 16 contraction tiles for projections
TW = 512              # token tile width (matmul free dim)
NT = T // TW          # 4 token tiles
JTN = T // P          # 16 key tiles of 128
SCALE = 1.0 / math.sqrt(HD)

FP = mybir.dt.float16
F32 = mybir.dt.float32

_CACHE = {}

# Set by kernel() after each run: bass_utils.BassKernelResults.
LAST_RESULT = None


def _build_bass():
    nc = bacc.Bacc("TRN2")

    # Host-packed layouts: partition dim first, then contiguous payload.
    xt = nc.dram_tensor("xt", [P, NT, KT, TW], FP, kind="ExternalInput")
    wq = nc.dram_tensor("wq", [P, KT, NH * HD], FP, kind="ExternalInput")
    wk = nc.dram_tensor("wk", [P, KT, HD], FP, kind="ExternalInput")
    wv = nc.dram_tensor("wv", [P, KT, HD], FP, kind="ExternalInput")
    wp = nc.dram_tensor("wp", [P, NH, C], FP, kind="ExternalInput")
    bq = nc.dram_tensor("bq", [P, NH], F32, kind="ExternalInput")
    bk = nc.dram_tensor("bk", [P, 1], F32, kind="ExternalInput")
    bv = nc.dram_tensor("bv", [HD], F32, kind="ExternalInput")
    mask = nc.dram_tensor("mask", [P, 2, P], FP, kind="ExternalInput")
    out = nc.dram_tensor("out", [T, C], FP, kind="ExternalOutput")

    out_r = out.ap().rearrange("(io p) o -> p io o", p=P)     # [128,16,2048]

    with tile.TileContext(nc) as tc, ExitStack() as ctx:
        consts = ctx.enter_context(tc.tile_pool(name="consts", bufs=1))
        xpool = ctx.enter_context(tc.tile_pool(name="xpool", bufs=2))
        espool = ctx.enter_context(tc.tile_pool(name="espool", bufs=4))
        mpool = ctx.enter_context(tc.tile_pool(name="mpool", bufs=2))
        opool = ctx.enter_context(tc.tile_pool(name="opool", bufs=2))
        # PSUM (8 banks): ps_s 2x[128,2,512] (4) for S pairs + q/k proj,
        # ps_y 1x[128,512] (1), ps_d 1x[128,512] (1) also l2-S and v-proj,
        # ps_o 1x[128,2,512] (2) for out-proj halves.
        ps_s = ctx.enter_context(tc.tile_pool(name="ps_s", bufs=2, space="PSUM"))
        ps_y = ctx.enter_context(tc.tile_pool(name="ps_y", bufs=1, space="PSUM"))
        ps_d = ctx.enter_context(tc.tile_pool(name="ps_d", bufs=1, space="PSUM"))
        ps_o = ctx.enter_context(tc.tile_pool(name="ps_o", bufs=1, space="PSUM"))

        # Weights needed first, loaded in k-chunks interleaved with the first
        # x tile so the first q matmul can start as early as possible.  x
        # rides the sync queue; weights ride the gpsimd queue so the startup
        # burst is split across two DMA rings.
        KC = 4  # k-chunks per load
        wq_sb = consts.tile([P, KT, NH * HD], FP)
        wk_sb = consts.tile([P, KT, HD], FP)
        wv_sb = consts.tile([P, KT, HD], FP)
        mask_sb = consts.tile([P, 2, P], FP)
        nc.gpsimd.dma_start(out=mask_sb, in_=mask.ap())
        xtile0 = xpool.tile([P, KT, TW], FP, tag="xt", name="xtile0")
        for c4 in range(KC):
            ks = slice(c4 * (KT // KC), (c4 + 1) * (KT // KC))
            nc.sync.dma_start(out=xtile0[:, ks], in_=xt.ap()[:, 0, ks])
            nc.gpsimd.dma_start(out=wq_sb[:, ks], in_=wq.ap()[:, ks])
            nc.gpsimd.dma_start(out=wk_sb[:, ks], in_=wk.ap()[:, ks])
            nc.gpsimd.dma_start(out=wv_sb[:, ks], in_=wv.ap()[:, ks])
        bq_sb = consts.tile([P, NH], F32)
        nc.gpsimd.dma_start(out=bq_sb, in_=bq.ap())
        bk_sb = consts.tile([P, 1], F32)
        nc.gpsimd.dma_start(out=bk_sb, in_=bk.ap())
        # bv broadcast across partitions (DRAM source allows partition step 0).
        bv_bc = consts.tile([P, HD], F32)
        bv_ap = bass.AP(tensor=bv.ap().tensor, offset=0, ap=[[0, P], [1, HD]])
        nc.gpsimd.dma_start(out=bv_bc, in_=bv_ap)
        ones_sb = consts.tile([P, P], FP)
        nc.vector.memset(ones_sb, 1.0)
        dummy_sb = consts.tile([P, TW], FP)
        nc.vector.memset(dummy_sb, 0.0)

        # PE warm-up: HAM un-throttles (1.2 -> 2.4 GHz) after ~3.4us of
        # sustained matmul activity.  Run throwaway matmuls while the input
        # DMAs land so the real matmuls start at full clock.
        ps_warm = ps_o.tile([P, 2, TW], F32, tag="pso", name="ps_warm")
        for w in range(10):
            nc.tensor.matmul(
                ps_warm[:, w % 2, :],
                lhsT=ones_sb,
                rhs=dummy_sb,
                start=True,
                stop=True,
            )

        # Persistent activations.
        qT = consts.tile([P, NH, T], FP)       # [d, h, i]
        kT = consts.tile([P, T], FP)           # [d, j]
        v_sb = consts.tile([P, JTN, HD], FP)   # [j_in, j_tile, d]
        yT = consts.tile([P, NH, T], FP)       # [d, h, i]

        # ---- Projections ----
        # Stream the contraction dim: per 4-k chunk, feed all accumulators
        # (4 q heads, k, 4 v token blocks) so compute starts as soon as the
        # first chunk of wq/xt lands instead of after the full 4MB.
        for n in range(NT):
            if n == 0:
                xtile = xtile0
            else:
                xtile = xpool.tile([P, KT, TW], FP, tag="xt", name=f"xtile{n}")
                for c4 in range(KC):
                    ks = slice(c4 * (KT // KC), (c4 + 1) * (KT // KC))
                    nc.sync.dma_start(out=xtile[:, ks], in_=xt.ap()[:, n, ks])
            psq01 = ps_s.tile([P, 2, TW], F32, tag="pss", name=f"psq01_{n}")
            psq23 = ps_s.tile([P, 2, TW], F32, tag="pss", name=f"psq23_{n}")
            psk = ps_y.tile([P, TW], F32, tag="psy", name=f"psk_{n}")
            for k in range(KT):
                st = k == 0
                sp = k == KT - 1
                for h in range(NH):
                    tgt = psq01 if h < 2 else psq23
                    nc.tensor.matmul(
                        tgt[:, h % 2, :],
                        lhsT=wq_sb[:, k, h * HD:(h + 1) * HD],
                        rhs=xtile[:, k, :],
                        start=st,
                        stop=sp,
                    )
                nc.tensor.matmul(
                    psk, lhsT=wk_sb[:, k, :], rhs=xtile[:, k, :], start=st, stop=sp
                )
            for h in range(NH):
                tgt = psq01 if h < 2 else psq23
                nc.vector.tensor_scalar(
                    out=qT[:, h, n * TW:(n + 1) * TW],
                    in0=tgt[:, h % 2, :],
                    scalar1=bq_sb[:, h:h + 1],
                    scalar2=None,
                    op0=mybir.AluOpType.add,
                )
            nc.vector.tensor_scalar(
                out=kT[:, n * TW:(n + 1) * TW],
                in0=psk,
                scalar1=bk_sb,
                scalar2=None,
                op0=mybir.AluOpType.add,
            )
            # v-projection: DMA-independent by now (q/k streamed the whole
            # xtile); two accumulators per ps_o tile, one bank each.
            for jp in range(TW // P // 2):
                psv = ps_o.tile([P, 2, TW], F32, tag="pso", name=f"psv_{n}_{jp}")
                for u in range(2):
                    js = jp * 2 + u
                    for k in range(KT):
                        nc.tensor.matmul(
                            psv[:, u, :HD],
                            lhsT=xtile[:, k, js * P:(js + 1) * P],
                            rhs=wv_sb[:, k, :],
                            start=(k == 0),
                            stop=(k == KT - 1),
                        )
                for u in range(2):
                    jt = n * (TW // P) + jp * 2 + u
                    nc.vector.tensor_tensor(
                        out=v_sb[:, jt, :],
                        in0=psv[:, u, :HD],
                        in1=bv_bc,
                        op=mybir.AluOpType.add,
                    )

        # Weights for the out-projection: load after projection work is
        # queued, on the gpsimd queue (idle once the small weights landed).
        wp_sb = consts.tile([P, NH, C], FP)
        nc.gpsimd.dma_start(out=wp_sb, in_=wp.ap())

        # ---- Attention with interleaved output projection ----
        # Out-proj for i-tile it is emitted *split* between the attention
        # heads of i-tile it+1: half right after the diagonal S matmuls
        # (covering the diag exp latency), half after the den matmul.  The
        # row-sum accumulation runs on the otherwise-idle gpsimd engine.
        def out_proj_half(ic, half, osb, pool=ps_o, ptag="pso"):
            pso = pool.tile([P, 2, TW], F32, tag=ptag, name=f"pso_{ic}_{half}")
            for h in range(NH):
                for u in range(2):
                    ot = half * 2 + u
                    nc.tensor.matmul(
                        pso[:, u, :],
                        lhsT=yT[:, h, ic * P:(ic + 1) * P],
                        rhs=wp_sb[:, h, ot * TW:(ot + 1) * TW],
                        start=(h == 0),
                        stop=(h == NH - 1),
                    )
            hsl = slice(half * 2 * TW, (half + 1) * 2 * TW)
            nc.any.tensor_copy(out=osb[:, hsl], in_=pso)
            if half == 1:
                nc.sync.dma_start(out=out_r[:, ic, :], in_=osb)

        for it in range(NT):
            isl = slice(it * TW, (it + 1) * TW)
            noff = 2 * it          # full-width off-diagonal key-tile pairs
            for h in range(NH):
                ic = (it - 1) * (TW // P) + h   # out-proj chunk to interleave
                osb = None
                if it > 0:
                    osb = opool.tile([P, C], FP, tag="osb", name=f"osb_{ic}")
                psy = ps_y.tile([P, TW], F32, tag="psy", name=f"psy_{it}_{h}")
                acc = mpool.tile([P, 2, TW], FP, tag="acc", name=f"acc_{it}_{h}")
                # --- off-diagonal pairs: full 512-wide, unmasked ---
                for pr in range(noff):
                    jt0 = 2 * pr
                    pss = ps_s.tile(
                        [P, 2, TW], F32, tag="pss", name=f"pss_{it}_{h}_{pr}"
                    )
                    for u in range(2):
                        nc.tensor.matmul(
                            pss[:, u, :],
                            lhsT=kT[:, (jt0 + u) * P:(jt0 + u + 1) * P],
                            rhs=qT[:, h, isl],
                            start=True,
                            stop=True,
                        )
                    es = espool.tile([P, 2, TW], FP, tag="es")
                    nc.scalar.activation(
                        out=es,
                        in_=pss,
                        func=mybir.ActivationFunctionType.Exp,
                        scale=SCALE,
                    )
                    if pr == 0:
                        nc.vector.tensor_copy(out=acc, in_=es)
                    else:
                        nc.vector.tensor_tensor(
                            out=acc, in0=acc, in1=es, op=mybir.AluOpType.add
                        )
                    for u in range(2):
                        nc.tensor.matmul(
                            psy,
                            lhsT=v_sb[:, jt0 + u, :],
                            rhs=es[:, u, :],
                            start=(jt0 + u == 0),
                            stop=False,
                        )
                # --- diagonal: 4 key tiles, live query ranges only ---
                # diagA psum pair: u0 <- l0 (full 512), u1 <- l1 (384-wide,
                # queries [128:512)) packed at [0:384) ++ l3 (128-wide,
                # queries [384:512)) at [384:512).  l2 (256-wide, queries
                # [256:512)) rides the ps_d bank ahead of the den matmul.
                jd = 4 * it        # first diagonal key tile
                diagA = ps_s.tile(
                    [P, 2, TW], F32, tag="pss", name=f"diagA_{it}_{h}"
                )
                psd_s = ps_d.tile([P, TW], F32, tag="psd", name=f"psl2_{it}_{h}")
                nc.tensor.matmul(
                    diagA[:, 0, :],
                    lhsT=kT[:, jd * P:(jd + 1) * P],
                    rhs=qT[:, h, isl],
                    start=True,
                    stop=True,
                )
                nc.tensor.matmul(
                    diagA[:, 1, 0:384],
                    lhsT=kT[:, (jd + 1) * P:(jd + 2) * P],
                    rhs=qT[:, h, it * TW + P:(it + 1) * TW],
                    start=True,
                    stop=True,
                )
                nc.tensor.matmul(
                    diagA[:, 1, 384:512],
                    lhsT=kT[:, (jd + 3) * P:(jd + 4) * P],
                    rhs=qT[:, h, it * TW + 3 * P:(it + 1) * TW],
                    start=True,
                    stop=True,
                )
                nc.tensor.matmul(
                    psd_s[:, 0:256],
                    lhsT=kT[:, (jd + 2) * P:(jd + 3) * P],
                    rhs=qT[:, h, it * TW + 2 * P:(it + 1) * TW],
                    start=True,
                    stop=True,
                )
                esd = espool.tile([P, 2, TW], FP, tag="es", name=f"esd_{it}_{h}")
                nc.scalar.activation(
                    out=esd,
                    in_=diagA,
                    func=mybir.ActivationFunctionType.Exp,
                    scale=SCALE,
                )
                es2 = espool.tile([P, 2, TW], FP, tag="es", name=f"es2_{it}_{h}")
                nc.scalar.activation(
                    out=es2[:, 0, 0:256],
                    in_=psd_s[:, 0:256],
                    func=mybir.ActivationFunctionType.Exp,
                    scale=SCALE,
                )
                # PE filler while ACT computes the diagonal exps.
                if it > 0:
                    out_proj_half(ic, 0, osb)
                # tril masks: l0 triangle at u0[0:128), l1 triangle at
                # u1[0:128) (one paired op), l3 triangle at u1[384:512),
                # l2 triangle at es2[0:128).
                nc.vector.tensor_mul(
                    esd[:, :, 0:P], esd[:, :, 0:P], mask_sb
                )
                nc.vector.tensor_mul(
                    esd[:, 1, 384:512], esd[:, 1, 384:512], mask_sb[:, 0, :]
                )
                nc.vector.tensor_mul(
                    es2[:, 0, 0:P], es2[:, 0, 0:P], mask_sb[:, 0, :]
                )
                # accumulate row sums (per-query) into the pair acc (gpsimd)
                if it == 0:
                    nc.vector.tensor_copy(out=acc[:, 0, :], in_=esd[:, 0, :])
                    nc.vector.memset(acc[:, 1, 0:P], 0.0)
                    nc.gpsimd.tensor_copy(
                        out=acc[:, 1, P:TW], in_=esd[:, 1, 0:384]
                    )
                else:
                    nc.vector.tensor_tensor(
                        out=acc[:, 0, :], in0=acc[:, 0, :], in1=esd[:, 0, :],
                        op=mybir.AluOpType.add,
                    )
                    nc.vector.tensor_tensor(
                        out=acc[:, 1, P:TW], in0=acc[:, 1, P:TW],
                        in1=esd[:, 1, 0:384], op=mybir.AluOpType.add,
                    )
                nc.vector.tensor_tensor(
                    out=acc[:, 0, 3 * P:TW], in0=acc[:, 0, 3 * P:TW],
                    in1=esd[:, 1, 384:512], op=mybir.AluOpType.add,
                )
                nc.vector.tensor_tensor(
                    out=acc[:, 1, 2 * P:TW], in0=acc[:, 1, 2 * P:TW],
                    in1=es2[:, 0, 0:256], op=mybir.AluOpType.add,
                )
                # --- AV for the diagonal ---
                # it==0: l0 (full width) first with start=True so no matmul
                # needs a sub-range start; the last one (l2) carries stop.
                # it>0: psy already initialized by off-diag jt=0; l0 goes
                # last, full width, with stop.
                if it == 0:
                    nc.tensor.matmul(
                        psy,
                        lhsT=v_sb[:, jd, :],
                        rhs=esd[:, 0, :],
                        start=True,
                        stop=False,
                    )
                nc.tensor.matmul(
                    psy[:, P:TW],
                    lhsT=v_sb[:, jd + 1, :],
                    rhs=esd[:, 1, 0:384],
                    start=False,
                    stop=False,
                )
                nc.tensor.matmul(
                    psy[:, 3 * P:TW],
                    lhsT=v_sb[:, jd + 3, :],
                    rhs=esd[:, 1, 384:512],
                    start=False,
                    stop=False,
                )
                nc.tensor.matmul(
                    psy[:, 2 * P:TW],
                    lhsT=v_sb[:, jd + 2, :],
                    rhs=es2[:, 0, 0:256],
                    start=False,
                    stop=(it == 0),
                )
                if it > 0:
                    nc.tensor.matmul(
                        psy,
                        lhsT=v_sb[:, jd, :],
                        rhs=esd[:, 0, :],
                        start=False,
                        stop=True,
                    )
                # --- denominator: fold pair slots, one ones-matmul ---
                accs = mpool.tile([P, TW], FP, tag="accs", name=f"accs_{it}_{h}")
                nc.vector.tensor_tensor(
                    out=accs, in0=acc[:, 0, :], in1=acc[:, 1, :],
                    op=mybir.AluOpType.add,
                )
                psd = ps_d.tile([P, TW], F32, tag="psd", name=f"psd_{it}_{h}")
                nc.tensor.matmul(
                    psd, lhsT=ones_sb, rhs=accs, start=True, stop=True
                )
                if it > 0:
                    out_proj_half(ic, 1, osb)
                rb = mpool.tile([P, TW], F32, tag="rb")
                nc.vector.reciprocal_approx_fast(out=rb, in_=psd)
                nc.vector.tensor_mul(yT[:, h, isl], psy, rb)
        # Tail chunks: attention is done, so the S-pair pool (2 slots) is free
        # and gives half-to-half pipelining.
        for h in range(NH):
            ic = (NT - 1) * (TW // P) + h
            osb = opool.tile([P, C], FP, tag="osb", name=f"osb_{ic}")
            for half in range(2):
                out_proj_half(ic, half, osb, pool=ps_s, ptag="pss")

    nc.compile()
    return nc


def _causal_mask_tiles():
    # [128, 2, 128] tril (key_in_tile <= query_in_block), both slots equal.
    j = np.arange(P)[:, None, None]
    i = np.arange(P)[None, None, :]
    return np.broadcast_to(j <= i, (P, 2, P)).astype(np.float16)


def kernel(x, Wkv, bkv, Wq, bq, Wp, bp):
    global LAST_RESULT
    x = np.asarray(x, np.float32)
    Wkv = np.asarray(Wkv, np.float32)
    bkv = np.asarray(bkv, np.float32)
    Wq = np.asarray(Wq, np.float32)
    bq = np.asarray(bq, np.float32)
    Wp = np.asarray(Wp, np.float32)
    bp = np.asarray(bp, np.float32)

    if "nc" not in _CACHE:
        _CACHE["nc"] = _build_bass()
    nc = _CACHE["nc"]

    mask = _causal_mask_tiles()
    CG = C // G  # 512 columns per kv head in the k/v halves of Wkv

    in_maps = []
    for b in range(B):
        # xt packed: [p, n, ko, tw] = x[b].T[ko*128+p, n*512+tw]
        xtb = x[b].T.astype(np.float16).reshape(KT, P, NT, TW)
        xt_packed = np.ascontiguousarray(xtb.transpose(1, 2, 0, 3))
        for g in range(HKV):
            heads = [g + HKV * u for u in range(NH)]  # h % HKV == g
            wq_g = np.concatenate(
                [Wq[:, h * HD:(h + 1) * HD] for h in heads], axis=1
            ).astype(np.float16)
            wq_p = np.ascontiguousarray(
                wq_g.reshape(KT, P, NH * HD).transpose(1, 0, 2)
            )
            bq_g = np.concatenate([bq[h * HD:(h + 1) * HD] for h in heads])
            bq_p = np.ascontiguousarray(
                bq_g.reshape(NH, P).T.astype(np.float32)
            )
            wp_g = np.concatenate(
                [Wp[h * HD:(h + 1) * HD, :] for h in heads], axis=0
            ).astype(np.float16)
            wp_p = np.ascontiguousarray(wp_g.reshape(NH, P, C).transpose(1, 0, 2))
            wk_g = Wkv[:, g * HD:(g + 1) * HD].astype(np.float16)
            wk_p = np.ascontiguousarray(wk_g.reshape(KT, P, HD).transpose(1, 0, 2))
            wv_g = Wkv[:, CG + g * HD:CG + (g + 1) * HD].astype(np.float16)
            wv_p = np.ascontiguousarray(wv_g.reshape(KT, P, HD).transpose(1, 0, 2))
            bk_g = np.ascontiguousarray(
                bkv[g * HD:(g + 1) * HD].reshape(P, 1), np.float32
            )
            bv_g = np.ascontiguousarray(
                bkv[CG + g * HD:CG + (g + 1) * HD], np.float32
            )
            in_maps.append(
                {
                    "xt": xt_packed,
                    "wq": wq_p,
                    "wk": wk_p,
                    "wv": wv_p,
                    "wp": wp_p,
                    "bq": bq_p,
                    "bk": bk_g,
                    "bv": bv_g,
                    "mask": mask,
                }
            )

    res = bass_utils.run_bass_kernel_spmd(nc, in_maps, core_ids=list(range(B * HKV)))
    LAST_RESULT = res

    out = np.zeros((B, T, C), np.float32)
    for b in range(B):
        acc = np.zeros((T, C), np.float32)
        for g in range(HKV):
            acc += res.results[b * HKV + g]["out"]
        out[b] = acc + bp[None, :]
    return out


# revision 8
# speedup vs baseline: 1.3823x; 1.0073x over previous
"""GQA causal self-attention on 8 Trainium2 NeuronCores.

Problem: B=2, T=2048, C=2048, H=16 query heads, HKV=4 kv heads, HD=128.
Sharding: core (b, g) for b in {0,1}, g in {0..3} owns batch b, kv head g,
and the 4 query heads h with h % 4 == g (reference's _expand_kv maps query
head h -> kv head h % HKV).  Each core computes its heads' attention output
and a partial output projection (its 512 rows of Wp); the host sums the 4
partials per batch and adds bp.  No cross-core communication on device.

All DRAM inputs are host-repacked to [128 partitions, ...contiguous]
layouts so every DMA chunk is >=4KB contiguous per partition (fast issue,
fast transfer).

Device math per core (all matmuls fp16 operands, fp32 PSUM accumulation):
  qT[d, t] = Wq_g.T @ x_b.T      (x is fed pre-transposed from host)
  kT[d, t] = Wk_g.T @ x_b.T
  v[t, d]  = x_b @ Wv_g          (lhsT = xT tiles)
  ST[j, i] = kT_j . qT_i         (j keys on partitions, i queries free)
  A = exp(ST / sqrt(HD)); causal: off-diagonal key tiles computed full
      width, the 4 diagonal key tiles of each i-tile computed only on
      their live query ranges (512/384/256/128 wide) with a tril mask on
      the single triangular 128-block each
  den[*, i] = ones-matmul over gpsimd-accumulated row sums
  yT[d, i] = (sum_j v[j, d] A[j, i]) / den[i]
  out[i, o] += yT.T @ Wp_g       (partial, fp16; host sums over g)
"""

import math
import os
from contextlib import ExitStack

import numpy as np

import concourse.bass as bass
import concourse.mybir as mybir
import concourse.tile as tile
from concourse import bacc, bass_utils

# The axon trace path needs antenv.axon_hooks; if the environment requests
# tracing but lacks the hook module, force tracing off instead of crashing.
if os.environ.get("BASS_TRACE"):
    try:
        import antenv.axon_hooks  # noqa: F401
    except ImportError:
        os.environ["BASS_NEVER_TRACE"] = "1"

# Problem shapes (hardcoded per contest rules).
B, T, C = 2, 2048, 2048
H, G = 16, 4
HKV = H // G          # 4 kv heads
HD = C // H           # 128 head dim
P = 128               # partitions
NH = H // HKV         # 4 local query heads per core
KT = C // P           # 16 contraction tiles for projections
TW = 512              # token tile width (matmul free dim)
NT = T // TW          # 4 token tiles
JTN = T // P          # 16 key tiles of 128
SCALE = 1.0 / math.sqrt(HD)

FP = mybir.dt.float16
F32 = mybir.dt.float32

_CACHE = {}

# Set by kernel() after each run: bass_utils.BassKernelResults.
LAST_RESULT = None


def _build_bass():
    nc = bacc.Bacc("TRN2")

    # Host-packed layouts: partition dim first, then contiguous payload.
    xt = nc.dram_tensor("xt", [P, NT, KT, TW], FP, kind="ExternalInput")
    wq = nc.dram_tensor("wq", [P, KT, NH * HD], FP, kind="ExternalInput")
    wk = nc.dram_tensor("wk", [P, KT, HD], FP, kind="ExternalInput")
    wv = nc.dram_tensor("wv", [P, KT, HD], FP, kind="ExternalInput")
    wp = nc.dram_tensor("wp", [P, NH, C], FP, kind="ExternalInput")
    bq = nc.dram_tensor("bq", [P, NH], F32, kind="ExternalInput")
    bk = nc.dram_tensor("bk", [P, 1], F32, kind="ExternalInput")
    bv = nc.dram_tensor("bv", [HD], F32, kind="ExternalInput")
    mask = nc.dram_tensor("mask", [P, 2, P], FP, kind="ExternalInput")
    out = nc.dram_tensor("out", [T, C], FP, kind="ExternalOutput")

    out_r = out.ap().rearrange("(io p) o -> p io o", p=P)     # [128,16,2048]

    with tile.TileContext(nc) as tc, ExitStack() as ctx:
        consts = ctx.enter_context(tc.tile_pool(name="consts", bufs=1))
        xpool = ctx.enter_context(tc.tile_pool(name="xpool", bufs=2))
        espool = ctx.enter_context(tc.tile_pool(name="espool", bufs=6))
        mpool = ctx.enter_context(tc.tile_pool(name="mpool", bufs=2))
        opool = ctx.enter_context(tc.tile_pool(name="opool", bufs=2))
        # PSUM (8 banks): ps_s 2x[128,2,512] (4) for S pairs + q/k proj,
        # ps_y 1x[128,512] (1), ps_d 1x[128,512] (1) also l2-S and v-proj,
        # ps_o 1x[128,2,512] (2) for out-proj halves.
        ps_s = ctx.enter_context(tc.tile_pool(name="ps_s", bufs=2, space="PSUM"))
        ps_y = ctx.enter_context(tc.tile_pool(name="ps_y", bufs=1, space="PSUM"))
        ps_d = ctx.enter_context(tc.tile_pool(name="ps_d", bufs=1, space="PSUM"))
        ps_o = ctx.enter_context(tc.tile_pool(name="ps_o", bufs=1, space="PSUM"))

        # Weights needed first, loaded in k-chunks interleaved with the first
        # x tile so the first q matmul can start as early as possible.  x
        # rides the sync queue; weights ride the gpsimd queue so the startup
        # burst is split across two DMA rings.
        KC = 4  # k-chunks per load
        wq_sb = consts.tile([P, KT, NH * HD], FP)
        wk_sb = consts.tile([P, KT, HD], FP)
        wv_sb = consts.tile([P, KT, HD], FP)
        mask_sb = consts.tile([P, 2, P], FP)
        nc.scalar.dma_start(out=mask_sb, in_=mask.ap())
        xtile0 = xpool.tile([P, KT, TW], FP, tag="xt", name="xtile0")
        for c4 in range(KC):
            ks = slice(c4 * (KT // KC), (c4 + 1) * (KT // KC))
            nc.sync.dma_start(out=xtile0[:, ks], in_=xt.ap()[:, 0, ks])
            nc.gpsimd.dma_start(out=wq_sb[:, ks], in_=wq.ap()[:, ks])
            nc.scalar.dma_start(out=wk_sb[:, ks], in_=wk.ap()[:, ks])
            nc.scalar.dma_start(out=wv_sb[:, ks], in_=wv.ap()[:, ks])
        bq_sb = consts.tile([P, NH], F32)
        nc.scalar.dma_start(out=bq_sb, in_=bq.ap())
        bk_sb = consts.tile([P, 1], F32)
        nc.scalar.dma_start(out=bk_sb, in_=bk.ap())
        # bv broadcast across partitions (DRAM source allows partition step 0).
        bv_bc = consts.tile([P, HD], F32)
        bv_ap = bass.AP(tensor=bv.ap().tensor, offset=0, ap=[[0, P], [1, HD]])
        nc.scalar.dma_start(out=bv_bc, in_=bv_ap)
        ones_sb = consts.tile([P, P], FP)
        nc.vector.memset(ones_sb, 1.0)
        dummy_sb = consts.tile([P, TW], FP)
        nc.vector.memset(dummy_sb, 0.0)

        # PE warm-up: HAM un-throttles (1.2 -> 2.4 GHz) after ~3.4us of
        # sustained matmul activity.  Run throwaway matmuls while the input
        # DMAs land so the real matmuls start at full clock.
        ps_warm = ps_o.tile([P, 2, TW], F32, tag="pso", name="ps_warm")
        for w in range(10):
            nc.tensor.matmul(
                ps_warm[:, w % 2, :],
                lhsT=ones_sb,
                rhs=dummy_sb,
                start=True,
                stop=True,
            )

        # Persistent activations.
        qT = consts.tile([P, NH, T], FP)       # [d, h, i]
        kT = consts.tile([P, T], FP)           # [d, j]
        v_sb = consts.tile([P, JTN, HD], FP)   # [j_in, j_tile, d]
        yT = consts.tile([P, NH, T], FP)       # [d, h, i]

        # ---- Projections ----
        # Stream the contraction dim: per 4-k chunk, feed all accumulators
        # (4 q heads, k, 4 v token blocks) so compute starts as soon as the
        # first chunk of wq/xt lands instead of after the full 4MB.
        for n in range(NT):
            if n == 0:
                xtile = xtile0
            else:
                xtile = xpool.tile([P, KT, TW], FP, tag="xt", name=f"xtile{n}")
                for c4 in range(KC):
                    ks = slice(c4 * (KT // KC), (c4 + 1) * (KT // KC))
                    nc.sync.dma_start(out=xtile[:, ks], in_=xt.ap()[:, n, ks])
            psq01 = ps_s.tile([P, 2, TW], F32, tag="pss", name=f"psq01_{n}")
            psq23 = ps_s.tile([P, 2, TW], F32, tag="pss", name=f"psq23_{n}")
            psk = ps_y.tile([P, TW], F32, tag="psy", name=f"psk_{n}")
            for k in range(KT):
                st = k == 0
                sp = k == KT - 1
                for h in range(NH):
                    tgt = psq01 if h < 2 else psq23
                    nc.tensor.matmul(
                        tgt[:, h % 2, :],
                        lhsT=wq_sb[:, k, h * HD:(h + 1) * HD],
                        rhs=xtile[:, k, :],
                        start=st,
                        stop=sp,
                    )
                nc.tensor.matmul(
                    psk, lhsT=wk_sb[:, k, :], rhs=xtile[:, k, :], start=st, stop=sp
                )
            for h in range(NH):
                tgt = psq01 if h < 2 else psq23
                nc.vector.tensor_scalar(
                    out=qT[:, h, n * TW:(n + 1) * TW],
                    in0=tgt[:, h % 2, :],
                    scalar1=bq_sb[:, h:h + 1],
                    scalar2=None,
                    op0=mybir.AluOpType.add,
                )
            nc.vector.tensor_scalar(
                out=kT[:, n * TW:(n + 1) * TW],
                in0=psk,
                scalar1=bk_sb,
                scalar2=None,
                op0=mybir.AluOpType.add,
            )
            # v-projection: DMA-independent by now (q/k streamed the whole
            # xtile); two accumulators per ps_o tile, one bank each.
            for jp in range(TW // P // 2):
                psv = ps_o.tile([P, 2, TW], F32, tag="pso", name=f"psv_{n}_{jp}")
                for u in range(2):
                    js = jp * 2 + u
                    for k in range(KT):
                        nc.tensor.matmul(
                            psv[:, u, :HD],
                            lhsT=xtile[:, k, js * P:(js + 1) * P],
                            rhs=wv_sb[:, k, :],
                            start=(k == 0),
                            stop=(k == KT - 1),
                        )
                for u in range(2):
                    jt = n * (TW // P) + jp * 2 + u
                    nc.vector.tensor_tensor(
                        out=v_sb[:, jt, :],
                        in0=psv[:, u, :HD],
                        in1=bv_bc,
                        op=mybir.AluOpType.add,
                    )

        # Weights for the out-projection: load after projection work is
        # queued, on the gpsimd queue (idle once the small weights landed).
        wp_sb = consts.tile([P, NH, C], FP)
        nc.scalar.dma_start(out=wp_sb, in_=wp.ap())

        # ---- Attention with interleaved output projection ----
        # Out-proj for i-tile it is emitted *split* between the attention
        # heads of i-tile it+1: half right after the diagonal S matmuls
        # (covering the diag exp latency), half after the den matmul.  The
        # row-sum accumulation runs on the otherwise-idle gpsimd engine.
        def out_proj_half(ic, half, osb, pool=ps_o, ptag="pso"):
            pso = pool.tile([P, 2, TW], F32, tag=ptag, name=f"pso_{ic}_{half}")
            for h in range(NH):
                for u in range(2):
                    ot = half * 2 + u
                    nc.tensor.matmul(
                        pso[:, u, :],
                        lhsT=yT[:, h, ic * P:(ic + 1) * P],
                        rhs=wp_sb[:, h, ot * TW:(ot + 1) * TW],
                        start=(h == 0),
                        stop=(h == NH - 1),
                    )
            hsl = slice(half * 2 * TW, (half + 1) * 2 * TW)
            nc.any.tensor_copy(out=osb[:, hsl], in_=pso)
            if half == 1:
                nc.sync.dma_start(out=out_r[:, ic, :], in_=osb)

        for it in range(NT):
            isl = slice(it * TW, (it + 1) * TW)
            noff = 2 * it          # full-width off-diagonal key-tile pairs
            jd = 4 * it            # first diagonal key tile
            for h in range(NH):
                ic = (it - 1) * (TW // P) + h   # out-proj chunk to interleave
                osb = None
                if it > 0:
                    osb = opool.tile([P, C], FP, tag="osb", name=f"osb_{ic}")
                psy = ps_y.tile([P, TW], F32, tag="psy", name=f"psy_{it}_{h}")
                acc = mpool.tile([P, 2, TW], FP, tag="acc", name=f"acc_{it}_{h}")
                es_list = {}

                def emit_S(pr):
                    # S pair pr -> psum, exp -> es, row-sum -> acc (DVE)
                    jt0 = 2 * pr
                    pss = ps_s.tile(
                        [P, 2, TW], F32, tag="pss", name=f"pss_{it}_{h}_{pr}"
                    )
                    for u in range(2):
                        nc.tensor.matmul(
                            pss[:, u, :],
                            lhsT=kT[:, (jt0 + u) * P:(jt0 + u + 1) * P],
                            rhs=qT[:, h, isl],
                            start=True,
                            stop=True,
                        )
                    es = espool.tile([P, 2, TW], FP, tag="es")
                    nc.scalar.activation(
                        out=es,
                        in_=pss,
                        func=mybir.ActivationFunctionType.Exp,
                        scale=SCALE,
                    )
                    if pr == 0:
                        nc.vector.tensor_copy(out=acc, in_=es)
                    else:
                        nc.vector.tensor_tensor(
                            out=acc, in0=acc, in1=es, op=mybir.AluOpType.add
                        )
                    es_list[pr] = es

                def emit_AV(pr):
                    jt0 = 2 * pr
                    es = es_list[pr]
                    for u in range(2):
                        nc.tensor.matmul(
                            psy,
                            lhsT=v_sb[:, jt0 + u, :],
                            rhs=es[:, u, :],
                            start=(jt0 + u == 0),
                            stop=False,
                        )

                def emit_Sdiag():
                    # diagA pair: u0 <- l0 (full 512); u1 <- l1 (384-wide,
                    # queries [128:512)) packed at [0:384) ++ l3 (128-wide,
                    # queries [384:512)) at [384:512).  l2 (256-wide,
                    # queries [256:512)): it==0 borrows psy[0:256] (psy is
                    # rewritten by AV l0 with start=True afterwards), it>0
                    # rides the ps_d bank ahead of the den matmul.
                    diagA = ps_s.tile(
                        [P, 2, TW], F32, tag="pss", name=f"diagA_{it}_{h}"
                    )
                    psd_s = ps_d.tile(
                        [P, TW], F32, tag="psd", name=f"psl2_{it}_{h}"
                    )
                    psl2 = psd_s[:, 0:256]
                    nc.tensor.matmul(
                        diagA[:, 0, :],
                        lhsT=kT[:, jd * P:(jd + 1) * P],
                        rhs=qT[:, h, isl],
                        start=True,
                        stop=True,
                    )
                    nc.tensor.matmul(
                        diagA[:, 1, 0:384],
                        lhsT=kT[:, (jd + 1) * P:(jd + 2) * P],
                        rhs=qT[:, h, it * TW + P:(it + 1) * TW],
                        start=True,
                        stop=True,
                    )
                    nc.tensor.matmul(
                        diagA[:, 1, 384:512],
                        lhsT=kT[:, (jd + 3) * P:(jd + 4) * P],
                        rhs=qT[:, h, it * TW + 3 * P:(it + 1) * TW],
                        start=True,
                        stop=True,
                    )
                    nc.tensor.matmul(
                        psl2,
                        lhsT=kT[:, (jd + 2) * P:(jd + 3) * P],
                        rhs=qT[:, h, it * TW + 2 * P:(it + 1) * TW],
                        start=True,
                        stop=True,
                    )
                    esd = espool.tile(
                        [P, 2, TW], FP, tag="es", name=f"esd_{it}_{h}"
                    )
                    nc.scalar.activation(
                        out=esd,
                        in_=diagA,
                        func=mybir.ActivationFunctionType.Exp,
                        scale=SCALE,
                    )
                    es2 = espool.tile(
                        [P, 2, TW], FP, tag="es", name=f"es2_{it}_{h}"
                    )
                    nc.scalar.activation(
                        out=es2[:, 0, 0:256],
                        in_=psl2,
                        func=mybir.ActivationFunctionType.Exp,
                        scale=SCALE,
                    )
                    # tril masks: l0 triangle at u0[0:128), l1 triangle at
                    # u1[0:128) (one paired op), l3 triangle at u1[384:512),
                    # l2 triangle at es2[0:128).
                    nc.vector.tensor_mul(
                        esd[:, :, 0:P], esd[:, :, 0:P], mask_sb
                    )
                    nc.vector.tensor_mul(
                        esd[:, 1, 384:512], esd[:, 1, 384:512], mask_sb[:, 0, :]
                    )
                    nc.vector.tensor_mul(
                        es2[:, 0, 0:P], es2[:, 0, 0:P], mask_sb[:, 0, :]
                    )
                    # row sums into the pair acc
                    if it == 0:
                        nc.vector.tensor_copy(out=acc[:, 0, :], in_=esd[:, 0, :])
                        nc.vector.memset(acc[:, 1, 0:P], 0.0)
                        nc.vector.tensor_copy(
                            out=acc[:, 1, P:TW], in_=esd[:, 1, 0:384]
                        )
                    else:
                        nc.vector.tensor_tensor(
                            out=acc[:, 0, :], in0=acc[:, 0, :],
                            in1=esd[:, 0, :], op=mybir.AluOpType.add,
                        )
                        nc.vector.tensor_tensor(
                            out=acc[:, 1, P:TW], in0=acc[:, 1, P:TW],
                            in1=esd[:, 1, 0:384], op=mybir.AluOpType.add,
                        )
                    nc.vector.tensor_tensor(
                        out=acc[:, 0, 3 * P:TW], in0=acc[:, 0, 3 * P:TW],
                        in1=esd[:, 1, 384:512], op=mybir.AluOpType.add,
                    )
                    nc.vector.tensor_tensor(
                        out=acc[:, 1, 2 * P:TW], in0=acc[:, 1, 2 * P:TW],
                        in1=es2[:, 0, 0:256], op=mybir.AluOpType.add,
                    )
                    es_list["d"] = esd
                    es_list["2"] = es2

                def emit_AVdiag():
                    esd = es_list["d"]
                    es2 = es_list["2"]
                    if it == 0:
                        # l0 first (start=True full width), l2 carries stop.
                        nc.tensor.matmul(
                            psy, lhsT=v_sb[:, jd, :], rhs=esd[:, 0, :],
                            start=True, stop=False,
                        )
                    nc.tensor.matmul(
                        psy[:, P:TW], lhsT=v_sb[:, jd + 1, :],
                        rhs=esd[:, 1, 0:384], start=False, stop=False,
                    )
                    nc.tensor.matmul(
                        psy[:, 3 * P:TW], lhsT=v_sb[:, jd + 3, :],
                        rhs=esd[:, 1, 384:512], start=False, stop=False,
                    )
                    nc.tensor.matmul(
                        psy[:, 2 * P:TW], lhsT=v_sb[:, jd + 2, :],
                        rhs=es2[:, 0, 0:256], start=False, stop=(it == 0),
                    )
                    if it > 0:
                        nc.tensor.matmul(
                            psy, lhsT=v_sb[:, jd, :], rhs=esd[:, 0, :],
                            start=False, stop=True,
                        )

                # --- emission schedule: keep the PE 2 psum slots ahead ---
                if noff == 0:
                    emit_Sdiag()
                    emit_AVdiag()
                else:
                    emit_S(0)
                    emit_S(1)
                    if it > 0:
                        out_proj_half(ic, 0, osb)
                    for p in range(noff):
                        emit_AV(p)
                        nxt = p + 2
                        if nxt < noff:
                            emit_S(nxt)
                        elif nxt == noff:
                            emit_Sdiag()
                    emit_AVdiag()
                # --- denominator: fold pair slots, one ones-matmul ---
                accs = mpool.tile([P, TW], FP, tag="accs", name=f"accs_{it}_{h}")
                nc.vector.tensor_tensor(
                    out=accs, in0=acc[:, 0, :], in1=acc[:, 1, :],
                    op=mybir.AluOpType.add,
                )
                psd = ps_d.tile([P, TW], F32, tag="psd", name=f"psd_{it}_{h}")
                nc.tensor.matmul(
                    psd, lhsT=ones_sb, rhs=accs, start=True, stop=True
                )
                if it > 0:
                    out_proj_half(ic, 1, osb)
                rb = mpool.tile([P, TW], F32, tag="rb")
                nc.vector.reciprocal_approx_fast(out=rb, in_=psd)
                nc.vector.tensor_mul(yT[:, h, isl], psy, rb)
        # Tail chunks: attention is done, so the S-pair pool (2 slots) is free
        # and gives half-to-half pipelining.
        for h in range(NH):
            ic = (NT - 1) * (TW // P) + h
            osb = opool.tile([P, C], FP, tag="osb", name=f"osb_{ic}")
            for half in range(2):
                out_proj_half(ic, half, osb, pool=ps_s, ptag="pss")

    nc.compile()
    return nc


def _causal_mask_tiles():
    # [128, 2, 128] tril (key_in_tile <= query_in_block), both slots equal.
    j = np.arange(P)[:, None, None]
    i = np.arange(P)[None, None, :]
    return np.broadcast_to(j <= i, (P, 2, P)).astype(np.float16)


def kernel(x, Wkv, bkv, Wq, bq, Wp, bp):
    global LAST_RESULT
    x = np.asarray(x, np.float32)
    Wkv = np.asarray(Wkv, np.float32)
    bkv = np.asarray(bkv, np.float32)
    Wq = np.asarray(Wq, np.float32)
    bq = np.asarray(bq, np.float32)
    Wp = np.asarray(Wp, np.float32)
    bp = np.asarray(bp, np.float32)

    if "nc" not in _CACHE:
        _CACHE["nc"] = _build_bass()
    nc = _CACHE["nc"]

    mask = _causal_mask_tiles()
    CG = C // G  # 512 columns per kv head in the k/v halves of Wkv

    in_maps = []
    for b in range(B):
        # xt packed: [p, n, ko, tw] = x[b].T[ko*128+p, n*512+tw]
        xtb = x[b].T.astype(np.float16).reshape(KT, P, NT, TW)
        xt_packed = np.ascontiguousarray(xtb.transpose(1, 2, 0, 3))
        for g in range(HKV):
            heads = [g + HKV * u for u in range(NH)]  # h % HKV == g
            wq_g = np.concatenate(
                [Wq[:, h * HD:(h + 1) * HD] for h in heads], axis=1
            ).astype(np.float16)
            wq_p = np.ascontiguousarray(
                wq_g.reshape(KT, P, NH * HD).transpose(1, 0, 2)
            )
            bq_g = np.concatenate([bq[h * HD:(h + 1) * HD] for h in heads])
            bq_p = np.ascontiguousarray(
                bq_g.reshape(NH, P).T.astype(np.float32)
            )
            wp_g = np.concatenate(
                [Wp[h * HD:(h + 1) * HD, :] for h in heads], axis=0
            ).astype(np.float16)
            wp_p = np.ascontiguousarray(wp_g.reshape(NH, P, C).transpose(1, 0, 2))
            wk_g = Wkv[:, g * HD:(g + 1) * HD].astype(np.float16)
            wk_p = np.ascontiguousarray(wk_g.reshape(KT, P, HD).transpose(1, 0, 2))
            wv_g = Wkv[:, CG + g * HD:CG + (g + 1) * HD].astype(np.float16)
            wv_p = np.ascontiguousarray(wv_g.reshape(KT, P, HD).transpose(1, 0, 2))
            bk_g = np.ascontiguousarray(
                bkv[g * HD:(g + 1) * HD].reshape(P, 1), np.float32
            )
            bv_g = np.ascontiguousarray(
                bkv[CG + g * HD:CG + (g + 1) * HD], np.float32
            )
            in_maps.append(
                {
                    "xt": xt_packed,
                    "wq": wq_p,
                    "wk": wk_p,
                    "wv": wv_p,
                    "wp": wp_p,
                    "bq": bq_p,
                    "bk": bk_g,
                    "bv": bv_g,
                    "mask": mask,
                }
            )

    res = bass_utils.run_bass_kernel_spmd(nc, in_maps, core_ids=list(range(B * HKV)))
    LAST_RESULT = res

    out = np.zeros((B, T, C), np.float32)
    for b in range(B):
        acc = np.zeros((T, C), np.float32)
        for g in range(HKV):
            acc += res.results[b * HKV + g]["out"]
        out[b] = acc + bp[None, :]
    return out
